# revision 5
# baseline (speedup 1.0000x reference)
"""Trainium2 Bass kernel: batched attention  out = softmax(Q K^T) V  (no 1/sqrt(d) scale).

Shapes (hardcoded): Q, K, V: [4, 16, 2048, 128] fp32 -> out [4, 16, 2048, 128] fp32.

Sharding: B*H = 64 heads, data-parallel across 8 NeuronCores (8 heads per core).

Per-head device algorithm (transpose-free matmul layout):
  Host pre-transposes Q, K to QT, KT = [D, N] per head (d-major), V stays [N, D].
  For each 128-wide key chunk c:
      S_T[c]  = KT[:, c].T @ QT            -> PSUM [128k, q]  (S transposed)
      E[c]    = exp(S_T[c])                -> SBUF   (ACT engine; no max-subtract
                                              needed: |S| <~ 70 fits fp32 exp)
      O_T    += V[c].T   @ E[c]            -> PSUM [128d, q]  (PSUM accumulate)
      l      += ones.T   @ E[c]            -> PSUM [1, q]     (row sums of exp)
  R = broadcast(l) via ones-outer-product matmul; r = 1/R (DVE reciprocal)
  O_sb = O_T * r (DVE) -> DMA out as O_T [D, N] per head; host transposes back.
"""

import sys

sys.path.insert(0, "/opt/trn_rl_repo")

import numpy as np

import concourse.bass as bass
import concourse.tile as tile
from concourse import bacc, mybir
from concourse.bass_utils import run_bass_kernel_spmd

B, H, N, D = 4, 16, 2048, 128
NCORES = 8
HPC = (B * H) // NCORES  # heads per core = 8
P = 128                  # partitions
NK = N // P              # key chunks per head = 16
QH = 2                   # q halves (1024 each) to fit PSUM
QHW = N // QH            # 1024
F32 = mybir.dt.float32


def build_nc():
    nc = bacc.Bacc(None, target_bir_lowering=False)

    qt_d = nc.dram_tensor("qt", [HPC, D, N], F32, kind="ExternalInput")
    kt_d = nc.dram_tensor("kt", [HPC, D, N], F32, kind="ExternalInput")
    v_d = nc.dram_tensor("v", [HPC, N, D], F32, kind="ExternalInput")
    ot_d = nc.dram_tensor("ot", [HPC, D, N], F32, kind="ExternalOutput")

    with tile.TileContext(nc) as tc:
        with (
            tc.tile_pool(name="const", bufs=1) as const_pool,
            tc.tile_pool(name="io", bufs=2) as io_pool,
            tc.tile_pool(name="e", bufs=4) as e_pool,
            tc.tile_pool(name="osb", bufs=2) as o_pool,
            tc.tile_pool(name="small", bufs=2) as small_pool,
            tc.tile_pool(name="ps_s", bufs=2, space="PSUM") as ps_s_pool,
            tc.tile_pool(name="ps_o", bufs=1, space="PSUM") as ps_o_pool,
            tc.tile_pool(name="ps_lr", bufs=1, space="PSUM") as ps_lr_pool,
        ):
            ones_col = const_pool.tile([P, 1], F32)  # sum weights: out row = col sums
            nc.vector.memset(ones_col[:], 1.0)
            ones_row = const_pool.tile([1, P], F32)  # broadcast weights
            nc.vector.memset(ones_row[:], 1.0)

            for h in range(HPC):
                qt = io_pool.tile([P, N], F32, tag="qt")
                nc.sync.dma_start(out=qt[:], in_=qt_d[h])
                kt = io_pool.tile([P, N], F32, tag="kt")
                nc.sync.dma_start(out=kt[:], in_=kt_d[h])
                # vt[p, c, d] = V[h, c*128 + p, d]
                vt3 = io_pool.tile([P, NK, P], F32, tag="vt")
                nc.sync.dma_start(
                    out=vt3[:], in_=v_d[h].rearrange("(c p) d -> p c d", p=P)
                )
                vt = vt3.rearrange("p c d -> p (c d)")

                for qh in range(QH):
                    q0 = qh * QHW
                    ps_o = ps_o_pool.tile([P, QHW], F32, tag="o")
                    ps_l = ps_lr_pool.tile([P, QHW], F32, tag="lr")
                    for c in range(NK):
                        ps_s = ps_s_pool.tile([P, QHW], F32, tag="s")
                        kc = kt[:, c * P:(c + 1) * P]
                        for j in range(2):
                            nc.tensor.matmul(
                                ps_s[:, j * 512:(j + 1) * 512],
                                kc,
                                qt[:, q0 + j * 512: q0 + (j + 1) * 512],
                                start=True,
                                stop=True,
                            )
                        e = e_pool.tile([P, QHW], F32, tag="e")
                        nc.scalar.activation(
                            e[:], ps_s[:], mybir.ActivationFunctionType.Exp
                        )
                        vc = vt[:, c * P:(c + 1) * P]
                        for j in range(2):
                            sl = slice(j * 512, (j + 1) * 512)
                            nc.tensor.matmul(
                                ps_o[:, sl],
                                vc,
                                e[:, sl],
                                start=(c == 0),
                                stop=(c == NK - 1),
                            )
                        for j in range(2):
                            sl = slice(j * 512, (j + 1) * 512)
                            nc.tensor.matmul(
                                ps_l[0:1, sl],
                                ones_col[:],
                                e[:, sl],
                                start=(c == 0),
                                stop=(c == NK - 1),
                            )
                    # l -> SBUF, broadcast to 128 partitions, reciprocal, scale O
                    l_sb = small_pool.tile([1, QHW], F32, tag="l")
                    nc.vector.tensor_copy(l_sb[:], ps_l[0:1, :])
                    ps_r = ps_lr_pool.tile([P, QHW], F32, tag="lr")
                    for j in range(2):
                        sl = slice(j * 512, (j + 1) * 512)
                        nc.tensor.matmul(
                            ps_r[:, sl], ones_row[:], l_sb[:, sl],
                            start=True, stop=True,
                        )
                    r_sb = small_pool.tile([P, QHW], F32, tag="r")
                    nc.vector.reciprocal(r_sb[:], ps_r[:])
                    o_sb = o_pool.tile([P, QHW], F32, tag="osb")
                    nc.vector.tensor_mul(o_sb[:], ps_o[:], r_sb[:])
                    nc.sync.dma_start(out=ot_d[h][:, q0: q0 + QHW], in_=o_sb[:])
    nc.finalize()
    return nc


def _prepare_in_maps(Q, K, V):
    Qf = np.asarray(Q, dtype=np.float32).reshape(B * H, N, D)
    Kf = np.asarray(K, dtype=np.float32).reshape(B * H, N, D)
    Vf = np.ascontiguousarray(np.asarray(V, dtype=np.float32).reshape(B * H, N, D))
    QT = np.ascontiguousarray(Qf.transpose(0, 2, 1))  # [64, D, N]
    KT = np.ascontiguousarray(Kf.transpose(0, 2, 1))
    in_maps = []
    for i in range(NCORES):
        s = slice(i * HPC, (i + 1) * HPC)
        in_maps.append({"qt": QT[s], "kt": KT[s], "v": Vf[s]})
    return in_maps


def run(Q, K, V, trace=False, **kwargs):
    nc = build_nc()
    in_maps = _prepare_in_maps(Q, K, V)
    res = run_bass_kernel_spmd(nc, in_maps, list(range(NCORES)), trace=trace, **kwargs)
    OT = np.concatenate([res.results[i]["ot"] for i in range(NCORES)], axis=0)
    out = OT.transpose(0, 2, 1).reshape(B, H, N, D)
    return np.ascontiguousarray(out), res


def kernel(Q, K, V):
    out, _ = run(Q, K, V, trace=False)
    return out


# revision 6
# speedup vs baseline: 2.4019x; 2.4019x over previous
"""Trainium2 Bass kernel: batched attention  out = softmax(Q K^T) V  (no 1/sqrt(d) scale).

Shapes (hardcoded): Q, K, V: [4, 16, 2048, 128] fp32 -> out [4, 16, 2048, 128] fp32.

Sharding: B*H = 64 heads, data-parallel across 8 NeuronCores (8 heads per core).

Per-head device algorithm (transpose-free matmul layout, bf16 PE with hi/lo
split for the accuracy-critical S = Q K^T):
  Host pre-transposes Q, K to [D, N] per head and splits each into bf16
  hi + lo parts (q = q1 + q2 exactly to ~16 mantissa bits). V is sent bf16.
  For each 128-wide key chunk c (S error ~2^-16: the dropped q2*k2 term):
      S_T[c]  = k1c.T @ q1 + k1c.T @ q2 + k2c.T @ q1   -> PSUM [128k, q] fp32
      E[c]    = exp(S_T[c])  (ACT; bf16 out; no max-subtract: |S| <~ 70
                fits fp32/bf16 exp range)
      O_T    += vc.T @ E[c]     (PSUM accumulate, fp32)
      l      += ones.T @ E[c]   (row sums of exp, PSUM row 0)
  r = approx-reciprocal(l) (DVE, ~2 ULP); broadcast r across partitions
  (GPSIMD); O_sb = O_T * r (DVE) -> DMA out as O_T [D, N]; host transposes.
"""

import sys

sys.path.insert(0, "/opt/trn_rl_repo")

import numpy as np
import ml_dtypes

import concourse.bass as bass
import concourse.tile as tile
from concourse import bacc, mybir
from concourse.bass_utils import run_bass_kernel_spmd

B, H, N, D = 4, 16, 2048, 128
NCORES = 8
HPC = (B * H) // NCORES  # heads per core = 8
P = 128                  # partitions
NK = N // P              # key chunks per head = 16
QH = 2                   # q halves (1024 each) to fit PSUM
QHW = N // QH            # 1024
F32 = mybir.dt.float32
BF16 = mybir.dt.bfloat16
BF16_NP = ml_dtypes.bfloat16


def build_nc():
    nc = bacc.Bacc(None, target_bir_lowering=False)

    q1_d = nc.dram_tensor("q1", [HPC, D, N], BF16, kind="ExternalInput")
    q2_d = nc.dram_tensor("q2", [HPC, D, N], BF16, kind="ExternalInput")
    k1_d = nc.dram_tensor("k1", [HPC, D, N], BF16, kind="ExternalInput")
    k2_d = nc.dram_tensor("k2", [HPC, D, N], BF16, kind="ExternalInput")
    v_d = nc.dram_tensor("v", [HPC, N, D], BF16, kind="ExternalInput")
    ot_d = nc.dram_tensor("ot", [HPC, D, N], F32, kind="ExternalOutput")

    with tile.TileContext(nc) as tc:
        with (
            tc.tile_pool(name="const", bufs=1) as const_pool,
            tc.tile_pool(name="io", bufs=2) as io_pool,
            tc.tile_pool(name="e", bufs=4) as e_pool,
            tc.tile_pool(name="osb", bufs=2) as o_pool,
            tc.tile_pool(name="small", bufs=2) as small_pool,
            tc.tile_pool(name="ps_s", bufs=2, space="PSUM") as ps_s_pool,
            tc.tile_pool(name="ps_o", bufs=1, space="PSUM") as ps_o_pool,
            tc.tile_pool(name="ps_l", bufs=1, space="PSUM") as ps_l_pool,
        ):
            ones_col = const_pool.tile([P, 1], BF16)  # sum weights
            nc.vector.memset(ones_col[:], 1.0)

            for h in range(HPC):
                q1t = io_pool.tile([P, N], BF16, tag="q1")
                nc.sync.dma_start(out=q1t[:], in_=q1_d[h])
                q2t = io_pool.tile([P, N], BF16, tag="q2")
                nc.sync.dma_start(out=q2t[:], in_=q2_d[h])
                k1t = io_pool.tile([P, N], BF16, tag="k1")
                nc.sync.dma_start(out=k1t[:], in_=k1_d[h])
                k2t = io_pool.tile([P, N], BF16, tag="k2")
                nc.sync.dma_start(out=k2t[:], in_=k2_d[h])
                # vt[p, c, d] = V[h, c*128 + p, d]
                vt3 = io_pool.tile([P, NK, P], BF16, tag="vt")
                nc.sync.dma_start(
                    out=vt3[:], in_=v_d[h].rearrange("(c p) d -> p c d", p=P)
                )
                vt = vt3.rearrange("p c d -> p (c d)")

                for qh in range(QH):
                    q0 = qh * QHW
                    ps_o = ps_o_pool.tile([P, QHW], F32, tag="o")
                    ps_l = ps_l_pool.tile([P, QHW], F32, tag="l")
                    for c in range(NK):
                        cs = slice(c * P, (c + 1) * P)
                        ps_s = ps_s_pool.tile([P, QHW], F32, tag="s")
                        # 3-term hi/lo split of S; group by lhsT for weight reuse
                        terms = [
                            (k1t[:, cs], q1t, False),
                            (k1t[:, cs], q2t, False),
                            (k2t[:, cs], q1t, True),
                        ]
                        for t, (kc, qt, last) in enumerate(terms):
                            for j in range(2):
                                sl = slice(j * 512, (j + 1) * 512)
                                nc.tensor.matmul(
                                    ps_s[:, sl],
                                    kc,
                                    qt[:, q0 + j * 512: q0 + (j + 1) * 512],
                                    start=(t == 0),
                                    stop=last,
                                )
                        e = e_pool.tile([P, QHW], BF16, tag="e")
                        nc.scalar.activation(
                            e[:], ps_s[:], mybir.ActivationFunctionType.Exp
                        )
                        for j in range(2):
                            sl = slice(j * 512, (j + 1) * 512)
                            nc.tensor.matmul(
                                ps_o[:, sl],
                                vt[:, cs],
                                e[:, sl],
                                start=(c == 0),
                                stop=(c == NK - 1),
                            )
                        for j in range(2):
                            sl = slice(j * 512, (j + 1) * 512)
                            nc.tensor.matmul(
                                ps_l[0:1, sl],
                                ones_col[:],
                                e[:, sl],
                                start=(c == 0),
                                stop=(c == NK - 1),
                            )
                    # r = 1/l (DVE approx, ~2 ULP), broadcast across partitions
                    # (GPSIMD), then O = O_T * r (DVE) and store.
                    r_sb = small_pool.tile([1, QHW], F32, tag="r")
                    scratch = small_pool.tile([1, QHW], F32, tag="rs")
                    nc.vector.reciprocal_approx_accurate(
                        r_sb[:], ps_l[0:1, :], scratch[:]
                    )
                    r_bc = small_pool.tile([P, QHW], F32, tag="rbc")
                    nc.gpsimd.partition_broadcast(r_bc[:], r_sb[:])
                    o_sb = o_pool.tile([P, QHW], F32, tag="osb")
                    nc.vector.tensor_mul(o_sb[:], ps_o[:], r_bc[:])
                    nc.sync.dma_start(out=ot_d[h][:, q0: q0 + QHW], in_=o_sb[:])
    nc.finalize()
    return nc


def _split_bf16_t(x):
    """[heads, N, D] fp32 -> transposed [heads, D, N] bf16 hi and lo parts."""
    xt = np.ascontiguousarray(x.transpose(0, 2, 1))
    hi = xt.astype(BF16_NP)
    lo = (xt - hi.astype(np.float32)).astype(BF16_NP)
    return hi, lo


def _prepare_in_maps(Q, K, V):
    Qf = np.asarray(Q, dtype=np.float32).reshape(B * H, N, D)
    Kf = np.asarray(K, dtype=np.float32).reshape(B * H, N, D)
    Vf = np.asarray(V, dtype=np.float32).reshape(B * H, N, D).astype(BF16_NP)
    q1, q2 = _split_bf16_t(Qf)
    k1, k2 = _split_bf16_t(Kf)
    in_maps = []
    for i in range(NCORES):
        s = slice(i * HPC, (i + 1) * HPC)
        in_maps.append(
            {"q1": q1[s], "q2": q2[s], "k1": k1[s], "k2": k2[s], "v": Vf[s]}
        )
    return in_maps


def run(Q, K, V, trace=False, **kwargs):
    nc = build_nc()
    in_maps = _prepare_in_maps(Q, K, V)
    res = run_bass_kernel_spmd(nc, in_maps, list(range(NCORES)), trace=trace, **kwargs)
    OT = np.concatenate([res.results[i]["ot"] for i in range(NCORES)], axis=0)
    out = OT.transpose(0, 2, 1).reshape(B, H, N, D)
    return np.ascontiguousarray(out), res


def kernel(Q, K, V):
    out, _ = run(Q, K, V, trace=False)
    return out
